# revision 3
# baseline (speedup 1.0000x reference)
"""Conditionally-modulated 3x3 conv (stride 1, pad 1) on 8 TRN2 NeuronCores.

Reference computation (per sample s):
    out[s] = conv2d(input[s] * cond[s, :, None, None], weight / sqrt(C*9)) + bias_mat[s]
with bias_mat[s, oc] = bias[(s*OUT_CH + oc) // B]  (torch repeat_interleave indexing).

Strategy: data-parallel over batch (16 samples -> 2 per core). Per core, a
1-D Winograd F(2,3) decomposition along the ROW (ky) axis turns each vertical
pair of output rows into 4 matmul accumulations (M1..M4) over (kx, ic) of
depth 768 each -- 12 PE cycles/output pixel instead of the direct conv's 18
(1.5x fewer PE cycles):

    y[2p]   = M1 + M2 + M3 + b      M1 = (x[2p-1] - x[2p+1]) * G1,  G1 = W[ky=0]
    y[2p+1] = M2 - M3 - M4 + b      M2 = (x[2p] + x[2p+1]) * G2,    G2 = (W[0]+W[1]+W[2])/2
                                    M3 = (x[2p+1] - x[2p]) * G3,    G3 = (W[0]-W[1]+W[2])/2
                                    M4 = (x[2p] - x[2p+2]) * G4,    G4 = W[ky=2]

All matmul operands are bf16 (input converted on host; weight transform folded
on host; per-sample condition scale folded into weights on device). The row
transforms run on the Vector engine (bf16 2x mode), the 4 PSUM banks are
evicted to SBUF by the Scalar engine, and the Vector engine recombines them
with the bias (scalar_tensor_tensor) into a bf16 output that the host upcasts
to fp32. End-to-end relative error ~5e-3 (gate 2e-2).

PE floor: 1536 matmuls x 512 rows x 0.4167ns = 328 us/core (vs 492 us direct).
"""

import math

import numpy as np
import ml_dtypes

import concourse.mybir as mybir
import concourse.tile as tile
from concourse import bacc
from concourse.alu_op_type import AluOpType
from concourse.bass_utils import run_bass_kernel_spmd

B, C, H, W = 16, 256, 128, 128
NCORES = 8
B_LOC = B // NCORES  # samples per core
KH = KW = 3
P = H // 2           # vertical output pair-rows per sample
SLAB_PR = 16         # pair-rows per input slab (32 output rows)
NSLAB = P // SLAB_PR
HS_PR = 8            # pair-rows per halfslab (transform granularity)
PRB = 4              # pair-rows per PSUM bank set
DC = W + 4           # D-tile columns: x col c -> D col c+2; pads at 1 and W+2
F32 = mybir.dt.float32
BF16 = mybir.dt.bfloat16
NPBF16 = ml_dtypes.bfloat16

_cache = {}


def _build(reps=1):
    """Build the per-core kernel. reps>1 (or "dyn") wraps the compute in a
    hardware loop repeating the identical work -- used only for wall-clock
    benching (the axon dispatch overhead is ~100ms, so single-shot timing is
    useless; differencing two rep counts isolates the per-iteration HW
    time)."""
    dyn = reps == "dyn"
    nc = bacc.Bacc("TRN2", target_bir_lowering=False, debug=False, num_devices=NCORES)

    x_d = nc.dram_tensor("x", [B_LOC, C, H, W], BF16, kind="ExternalInput").ap()
    # w[p, icb, m*3+kx, oc] = Gm[icb*128+p, kx, oc] (weight transform + scale)
    w_d = nc.dram_tensor("w", [128, 2, 12, C], BF16, kind="ExternalInput").ap()
    # cw[p, s, 0:2] = cond for ic blocks; cw[p, s, 2:4] = bias for oc blocks
    cw_d = nc.dram_tensor("cw", [128, B_LOC, 4], F32, kind="ExternalInput").ap()
    zr_d = nc.dram_tensor("zr", [128, 2, 1, W], BF16, kind="ExternalInput").ap()
    zc_d = nc.dram_tensor("zc", [128, 4, HS_PR, 1], BF16, kind="ExternalInput").ap()
    if dyn:
        r_d = nc.dram_tensor("r", [1, 1], mybir.dt.uint32, kind="ExternalInput").ap()
    y_d = nc.dram_tensor("y", [B_LOC, C, H, W], BF16, kind="ExternalOutput").ap()

    with tile.TileContext(nc) as tc:
        with (
            tc.tile_pool(name="const", bufs=1) as const_pool,
            tc.tile_pool(name="wsp", bufs=2) as ws_pool,
            tc.tile_pool(name="tp", bufs=8) as t_pool,
            tc.tile_pool(name="ep", bufs=4) as e_pool,
            tc.tile_pool(name="op", bufs=4) as o_pool,
            tc.tile_pool(name="ps", bufs=8, space="PSUM") as ps_pool,
        ):
            w_base = const_pool.tile([128, 2, 12, C], BF16)
            nc.sync.dma_start(w_base[:], w_d[:])
            cw = const_pool.tile([128, B_LOC, 4], F32)
            nc.sync.dma_start(cw[:], cw_d[:])
            zr = const_pool.tile([128, 2, 1, W], BF16)
            nc.sync.dma_start(zr[:], zr_d[:])

            # Persistent padded-input row slabs (34 rows: 32 + one halo row
            # each side) and transformed-row slabs D (cols 2..W+1 live, cols
            # 1 and W+2 are zero pads for the kx window; startup-zeroed).
            xp_bufs = [
                const_pool.tile([128, 2, 2 * SLAB_PR + 2, W], BF16, name=f"xpb{i}")
                for i in range(2)
            ]
            d_bufs = [
                const_pool.tile([128, 2, 4, HS_PR, DC], BF16, name=f"db{i}")
                for i in range(2)
            ]
            for db in d_bufs:
                for icb in range(2):
                    nc.sync.dma_start(db[:, icb, :, :, 1:2], zc_d[:])
                    nc.sync.dma_start(db[:, icb, :, :, W + 2 : W + 3], zc_d[:])

            import contextlib

            if dyn:
                r_sb = const_pool.tile([1, 1], mybir.dt.uint32)
                nc.sync.dma_start(r_sb[:], r_d[:])
                with tc.tile_critical():
                    n_iter = nc.values_load(
                        r_sb[0:1, 0:1],
                        min_val=0,
                        max_val=1 << 20,
                        skip_runtime_bounds_check=True,
                    )
                loop_cm = tc.For_i(0, n_iter, 1)
            elif reps > 1:
                loop_cm = tc.For_i(0, reps, 1)
            else:
                loop_cm = contextlib.nullcontext()
            with loop_cm:
                _emit_compute(nc, tc, ws_pool, t_pool, e_pool, o_pool, ps_pool,
                              x_d, y_d, cw, w_base, xp_bufs, d_bufs, zr)

    nc.compile()
    return nc


def _emit_xdma(nc, xp_bufs, x_d, zr, s, k):
    """DMA input rows for slab k (output rows 32k..32k+31, input rows
    32k-1..32k+32 with zero halos at the image edges)."""
    xp = xp_bufs[k % 2]
    y0 = 2 * SLAB_PR * k
    in_lo = max(y0 - 1, 0)
    in_hi = min(y0 + 2 * SLAB_PR + 1, H)
    dst_lo = in_lo - (y0 - 1)
    for icb in range(2):
        nc.sync.dma_start(
            xp[:, icb, dst_lo : dst_lo + (in_hi - in_lo), :],
            x_d[s, icb * 128 : (icb + 1) * 128, in_lo:in_hi, :],
        )
    if k == 0:
        nc.sync.dma_start(xp[:, :, 0:1, :], zr[:])
    if k == NSLAB - 1:
        nc.sync.dma_start(xp[:, :, 2 * SLAB_PR + 1 : 2 * SLAB_PR + 2, :], zr[:])


def _emit_transform(nc, xp_bufs, d_bufs, g):
    """Row transform for halfslab g (8 pair-rows): D1..D4 from strided row
    combinations of the padded slab. Everything bf16 -> DVE 2x mode."""
    k, h = divmod(g, 2)
    xp = xp_bufs[k % 2]
    dt = d_bufs[g % 2]
    rb = 2 * HS_PR * h
    n = 2 * HS_PR - 1  # stride-2 slice of HS_PR rows: end = start + 2*HS_PR - 1
    for icb in range(2):
        x0 = xp[:, icb, rb + 0 : rb + n + 0 : 2, :]
        x1 = xp[:, icb, rb + 1 : rb + n + 1 : 2, :]
        x2 = xp[:, icb, rb + 2 : rb + n + 2 : 2, :]
        x3 = xp[:, icb, rb + 3 : rb + n + 3 : 2, :]
        nc.vector.tensor_sub(dt[:, icb, 0, :, 2 : W + 2], x0, x2)
        nc.vector.tensor_add(dt[:, icb, 1, :, 2 : W + 2], x1, x2)
        nc.vector.tensor_sub(dt[:, icb, 2, :, 2 : W + 2], x2, x1)
        nc.vector.tensor_sub(dt[:, icb, 3, :, 2 : W + 2], x1, x3)


def _emit_sets(nc, t_pool, e_pool, o_pool, ps_pool, y_d, cw, w_s, d_bufs, s, g):
    """Matmul + eviction for halfslab g: 4 bank-sets (2 ocb x 2 prb), each
    4 PSUM banks M1..M4 of [128oc, 4pr, W] accumulating 6 matmuls (3kx x
    2icb), evicted via ScalarE copies and recombined on DVE."""
    k, h = divmod(g, 2)
    dt = d_bufs[g % 2]
    for ocb in range(2):
        bias_ap = cw[:, s, 2 + ocb : 3 + ocb]
        for prb in range(2):
            pl0 = PRB * prb
            banks = [ps_pool.tile([128, PRB, W], F32, name="ps") for _ in range(4)]
            for m in range(4):
                t = 0
                for icb in range(2):
                    for kx in range(3):
                        nc.tensor.matmul(
                            banks[m][:],
                            w_s[:, icb, m * 3 + kx, ocb * 128 : (ocb + 1) * 128],
                            dt[:, icb, m, pl0 : pl0 + PRB, kx + 1 : kx + 1 + W],
                            start=(t == 0),
                            stop=(t == 5),
                        )
                        t += 1
            ts = [t_pool.tile([128, PRB, W], BF16, name="t") for _ in range(4)]
            for m in range(4):
                nc.scalar.copy(ts[m][:], banks[m][:])
            e = e_pool.tile([128, PRB, W], BF16, name="e")
            o = e_pool.tile([128, PRB, W], BF16, name="o")
            ot = o_pool.tile([128, 2 * PRB, W], BF16, name="ot")
            nc.vector.scalar_tensor_tensor(
                e[:], ts[0][:], bias_ap, ts[1][:], AluOpType.add, AluOpType.add
            )
            nc.vector.tensor_add(ot[:, 0 : 2 * PRB : 2, :], e[:], ts[2][:])
            nc.vector.scalar_tensor_tensor(
                o[:], ts[1][:], bias_ap, ts[2][:], AluOpType.add, AluOpType.subtract
            )
            nc.vector.tensor_sub(ot[:, 1 : 2 * PRB : 2, :], o[:], ts[3][:])
            r0 = 2 * (SLAB_PR * k + HS_PR * h + pl0)
            nc.sync.dma_start(
                y_d[s, ocb * 128 : (ocb + 1) * 128, r0 : r0 + 2 * PRB, :],
                ot[:],
            )


def _emit_compute(nc, tc, ws_pool, t_pool, e_pool, o_pool, ps_pool,
                  x_d, y_d, cw, w_base, xp_bufs, d_bufs, zr):
    ng = 2 * NSLAB
    for s in range(B_LOC):
        # fold this sample's condition scale into the transformed weights
        w_s = ws_pool.tile([128, 2, 12, C], BF16, name="w_s")
        for icb in range(2):
            nc.vector.tensor_scalar_mul(
                w_s[:, icb], w_base[:, icb], cw[:, s, icb : icb + 1]
            )
        _emit_xdma(nc, xp_bufs, x_d, zr, s, 0)
        _emit_transform(nc, xp_bufs, d_bufs, 0)
        for g in range(ng):
            k = g // 2
            if g % 2 == 0 and k + 1 < NSLAB:
                _emit_xdma(nc, xp_bufs, x_d, zr, s, k + 1)
            if g + 1 < ng:
                _emit_transform(nc, xp_bufs, d_bufs, g + 1)
            _emit_sets(nc, t_pool, e_pool, o_pool, ps_pool, y_d, cw, w_s,
                       d_bufs, s, g)


def _get_nc():
    if "nc" not in _cache:
        _cache["nc"] = _build()
    return _cache["nc"]


def _make_in_maps(inputs):
    input = np.ascontiguousarray(np.asarray(inputs["input"], dtype=np.float32))
    cond = np.asarray(inputs["condition_feature"], dtype=np.float32).reshape(B, C)
    weight = np.asarray(inputs["weight"], dtype=np.float32)
    bias = np.asarray(inputs["bias"], dtype=np.float32)

    scale = 1.0 / math.sqrt(C * KH * KW)
    ws = weight * scale  # [oc, ic, ky, kx]
    W0, W1, W2 = ws[:, :, 0, :], ws[:, :, 1, :], ws[:, :, 2, :]
    G = np.stack([W0, (W0 + W1 + W2) / 2, (W0 - W1 + W2) / 2, W2])  # [m, oc, ic, kx]
    # [m, oc, ic, kx] -> [p, icb, m*3+kx, oc]
    w_host = np.ascontiguousarray(
        G.transpose(2, 0, 3, 1)          # [ic, m, kx, oc]
        .reshape(2, 128, 12, C)
        .transpose(1, 0, 2, 3)
        .astype(NPBF16)
    )
    bias_mat = np.repeat(bias, B).reshape(B, C)  # [s, oc]

    x_bf = input.astype(NPBF16)
    zr = np.zeros((128, 2, 1, W), dtype=NPBF16)
    zc = np.zeros((128, 4, HS_PR, 1), dtype=NPBF16)

    in_maps = []
    for c in range(NCORES):
        sl = slice(c * B_LOC, (c + 1) * B_LOC)
        cwm = np.empty((128, B_LOC, 4), dtype=np.float32)
        cond_c = cond[sl]  # [B_LOC, C]
        bias_c = bias_mat[sl]
        for s in range(B_LOC):
            cwm[:, s, 0] = cond_c[s, 0:128]
            cwm[:, s, 1] = cond_c[s, 128:256]
            cwm[:, s, 2] = bias_c[s, 0:128]
            cwm[:, s, 3] = bias_c[s, 128:256]
        in_maps.append(
            {"x": x_bf[sl], "w": w_host, "cw": cwm, "zr": zr, "zc": zc}
        )
    return in_maps


def kernel(input, condition_feature, weight, bias):
    in_maps = _make_in_maps(
        {
            "input": input,
            "condition_feature": condition_feature,
            "weight": weight,
            "bias": bias,
        }
    )
    nc = _get_nc()
    res = run_bass_kernel_spmd(nc, in_maps, list(range(NCORES)))
    out = np.concatenate([res.results[c]["y"] for c in range(NCORES)], axis=0)
    return np.ascontiguousarray(out.astype(np.float32))


if __name__ == "__main__":
    rng = np.random.default_rng(0)
    inputs = {
        "input": rng.standard_normal((B, C, H, W), dtype=np.float32),
        "condition_feature": rng.random((B, 1, C, 1, 1), dtype=np.float32),
        "weight": rng.standard_normal((C, C, KH, KW), dtype=np.float32),
        "bias": rng.standard_normal((C,), dtype=np.float32) * 0.1,
    }
    out = kernel(**inputs)
    print("out", out.shape, out.dtype, float(np.abs(out).max()))


# revision 12
# speedup vs baseline: 1.1839x; 1.1839x over previous
"""Conditionally-modulated 3x3 conv (stride 1, pad 1) on 8 TRN2 NeuronCores.

Reference computation (per sample s):
    out[s] = conv2d(input[s] * cond[s, :, None, None], weight / sqrt(C*9)) + bias_mat[s]
with bias_mat[s, oc] = bias[(s*OUT_CH + oc) // B]  (torch repeat_interleave indexing).

Strategy: data-parallel over batch (16 samples -> 2 per core). Per core, a
1-D Winograd F(2,3) decomposition along the ROW (ky) axis turns each vertical
pair of output rows into 4 matmul accumulations (M1..M4) over (kx, ic) of
depth 768 each -- 12 PE cycles/output pixel instead of the direct conv's 18
(1.5x fewer PE cycles):

    y[2p]   = M1 + M2 + M3 + b      M1 = (x[2p-1] - x[2p+1]) * G1,  G1 = W[ky=0]
    y[2p+1] = M2 - M3 - M4 + b      M2 = (x[2p] + x[2p+1]) * G2,    G2 = (W[0]+W[1]+W[2])/2
                                    M3 = (x[2p+1] - x[2p]) * G3,    G3 = (W[0]-W[1]+W[2])/2
                                    M4 = (x[2p] - x[2p+2]) * G4,    G4 = W[ky=2]

All matmul operands are bf16 (input converted on host; weight transform folded
on host; per-sample condition scale folded into weights on device). The row
transforms run on the Vector engine (bf16 2x mode), the 4 PSUM banks are
evicted to SBUF by the Scalar engine, and the Vector engine recombines them
with the bias (scalar_tensor_tensor) into a bf16 output that the host upcasts
to fp32. End-to-end relative error 5.2e-3 (gate 2e-2).

PE floor: 1536 matmuls x 512 rows x 0.4167ns = 328 us/core (vs 492 us direct
conv at fp32r). CoreSim (calibrated within 4% on the direct-conv baseline):
353 us. Cold-burst HW measurements: ~380 us vs the same protocol's ~640 us
for the 501 us-graded baseline. Engine budget per core: PE 328 us (bound),
DVE ~160 us, ScalarE ~146 us, DMA ~34 MB (~110 us) -- all hidden under PE.

Rejected alternatives (measured): fp8 DoubleRow needs both operands e4m3
(~3.7e-2 err, fails the gate; any hi/lo compensation costs >= 2 slots/MAC =
fp32r speed); F(3,3) cuts PE to 281 us but its 5-product transforms push the
DVE past the PE (CoreSim 430 us); 2-D Winograd is transform-bound worse.
"""

import math

import numpy as np
import ml_dtypes

import concourse.mybir as mybir
import concourse.tile as tile
from concourse import bacc
from concourse.alu_op_type import AluOpType
from concourse.bass_utils import run_bass_kernel_spmd

B, C, H, W = 16, 256, 128, 128
NCORES = 8
B_LOC = B // NCORES  # samples per core
KH = KW = 3
P = H // 2           # vertical output pair-rows per sample
SLAB_PR = 16         # pair-rows per input slab (32 output rows)
NSLAB = P // SLAB_PR
HS_PR = 8            # pair-rows per halfslab (transform granularity)
PRB = 4              # pair-rows per PSUM bank set
DC = W + 4           # D-tile columns: x col c -> D col c+2; pads at 1 and W+2
F32 = mybir.dt.float32
BF16 = mybir.dt.bfloat16
NPBF16 = ml_dtypes.bfloat16

_cache = {}

# Ablation switches (benchmarking only): subset of
# {"transforms", "matmuls", "act", "dve_evict", "out_dma", "in_dma"}.
ABLATE = set()


def _build(reps=1):
    """Build the per-core kernel. reps>1 (or "dyn") wraps the compute in a
    hardware loop repeating the identical work -- used only for wall-clock
    benching (the axon dispatch overhead is ~100ms, so single-shot timing is
    useless; differencing two rep counts isolates the per-iteration HW
    time)."""
    dyn = reps == "dyn"
    nc = bacc.Bacc("TRN2", target_bir_lowering=False, debug=False, num_devices=NCORES)

    x_d = nc.dram_tensor("x", [B_LOC, C, H, W], BF16, kind="ExternalInput").ap()
    # w[p, icb, m*3+kx, oc] = Gm[icb*128+p, kx, oc] (weight transform + scale)
    w_d = nc.dram_tensor("w", [128, 2, 12, C], BF16, kind="ExternalInput").ap()
    # cw[p, s, 0:2] = cond for ic blocks; cw[p, s, 2:4] = bias for oc blocks
    cw_d = nc.dram_tensor("cw", [128, B_LOC, 4], F32, kind="ExternalInput").ap()
    zr_d = nc.dram_tensor("zr", [128, 2, 1, W], BF16, kind="ExternalInput").ap()
    zc_d = nc.dram_tensor("zc", [128, 4, HS_PR, 1], BF16, kind="ExternalInput").ap()
    if dyn:
        r_d = nc.dram_tensor("r", [1, 1], mybir.dt.uint32, kind="ExternalInput").ap()
    y_d = nc.dram_tensor("y", [B_LOC, C, H, W], BF16, kind="ExternalOutput").ap()

    with tile.TileContext(nc) as tc:
        with (
            tc.tile_pool(name="const", bufs=1) as const_pool,
            tc.tile_pool(name="wsp", bufs=2) as ws_pool,
            tc.tile_pool(name="tp", bufs=8) as t_pool,
            tc.tile_pool(name="ep", bufs=4) as e_pool,
            tc.tile_pool(name="op", bufs=4) as o_pool,
            tc.tile_pool(name="ps", bufs=8, space="PSUM") as ps_pool,
        ):
            w_base = const_pool.tile([128, 2, 12, C], BF16)
            nc.sync.dma_start(w_base[:], w_d[:])
            cw = const_pool.tile([128, B_LOC, 4], F32)
            nc.sync.dma_start(cw[:], cw_d[:])
            zr = const_pool.tile([128, 2, 1, W], BF16)
            nc.sync.dma_start(zr[:], zr_d[:])

            # Persistent padded-input row slabs (34 rows: 32 + one halo row
            # each side) and transformed-row slabs D (cols 2..W+1 live, cols
            # 1 and W+2 are zero pads for the kx window; startup-zeroed).
            xp_bufs = [
                const_pool.tile([128, 2, 2 * SLAB_PR + 2, W], BF16, name=f"xpb{i}")
                for i in range(2)
            ]
            d_bufs = [
                const_pool.tile([128, 2, 4, HS_PR, DC], BF16, name=f"db{i}")
                for i in range(2)
            ]
            for db in d_bufs:
                for icb in range(2):
                    nc.sync.dma_start(db[:, icb, :, :, 1:2], zc_d[:])
                    nc.sync.dma_start(db[:, icb, :, :, W + 2 : W + 3], zc_d[:])

            import contextlib

            if dyn:
                r_sb = const_pool.tile([1, 1], mybir.dt.uint32)
                nc.sync.dma_start(r_sb[:], r_d[:])
                with tc.tile_critical():
                    n_iter = nc.values_load(
                        r_sb[0:1, 0:1],
                        min_val=0,
                        max_val=1 << 20,
                        skip_runtime_bounds_check=True,
                    )
                loop_cm = tc.For_i(0, n_iter, 1)
            elif reps > 1:
                loop_cm = tc.For_i(0, reps, 1)
            else:
                loop_cm = contextlib.nullcontext()
            with loop_cm:
                _emit_compute(nc, tc, ws_pool, t_pool, e_pool, o_pool, ps_pool,
                              x_d, y_d, cw, w_base, xp_bufs, d_bufs, zr)

    nc.compile()
    return nc


def _emit_xdma(nc, xp_bufs, x_d, zr, s, k):
    """DMA input rows for slab k (output rows 32k..32k+31, input rows
    32k-1..32k+32 with zero halos at the image edges)."""
    if "in_dma" in ABLATE:
        return
    xp = xp_bufs[k % 2]
    y0 = 2 * SLAB_PR * k
    in_lo = max(y0 - 1, 0)
    in_hi = min(y0 + 2 * SLAB_PR + 1, H)
    dst_lo = in_lo - (y0 - 1)
    # split each slab load at the halfslab boundary so the first halfslab's
    # transforms can start as soon as its rows land
    mid = y0 + 2 * HS_PR + 1  # first row needed only by the second halfslab
    for lo, hi in ((in_lo, mid), (mid, in_hi)):
        dl = lo - (y0 - 1)
        for icb in range(2):
            nc.sync.dma_start(
                xp[:, icb, dl : dl + (hi - lo), :],
                x_d[s, icb * 128 : (icb + 1) * 128, lo:hi, :],
            )
    if k == 0:
        nc.sync.dma_start(xp[:, :, 0:1, :], zr[:])
    if k == NSLAB - 1:
        nc.sync.dma_start(xp[:, :, 2 * SLAB_PR + 1 : 2 * SLAB_PR + 2, :], zr[:])


def _emit_transform(nc, xp_bufs, d_bufs, t):
    """Row transform for halfslab t (8 pair-rows): D1..D4 from strided row
    combinations of the padded slab. Everything bf16 -> DVE 2x mode."""
    if "transforms" in ABLATE:
        return
    g = t % (2 * NSLAB)
    k, h = divmod(g, 2)
    xp = xp_bufs[k % 2]
    dt = d_bufs[t % 2]
    rb = 2 * HS_PR * h
    n = 2 * HS_PR - 1  # stride-2 slice of HS_PR rows: end = start + 2*HS_PR - 1
    for icb in range(2):
        x0 = xp[:, icb, rb + 0 : rb + n + 0 : 2, :]
        x1 = xp[:, icb, rb + 1 : rb + n + 1 : 2, :]
        x2 = xp[:, icb, rb + 2 : rb + n + 2 : 2, :]
        x3 = xp[:, icb, rb + 3 : rb + n + 3 : 2, :]
        nc.vector.tensor_sub(dt[:, icb, 0, :, 2 : W + 2], x0, x2)
        nc.vector.tensor_add(dt[:, icb, 1, :, 2 : W + 2], x1, x2)
        nc.vector.tensor_sub(dt[:, icb, 2, :, 2 : W + 2], x2, x1)
        nc.vector.tensor_sub(dt[:, icb, 3, :, 2 : W + 2], x1, x3)


def _emit_sets(nc, t_pool, e_pool, o_pool, ps_pool, y_d, cw, w_s, d_bufs, s, t):
    """Matmul + eviction for halfslab t: 4 bank-sets (2 ocb x 2 prb), each
    4 PSUM banks M1..M4 of [128oc, 4pr, W] accumulating 6 matmuls (3kx x
    2icb), evicted via ScalarE copies and recombined on DVE."""
    g = t % (2 * NSLAB)
    k, h = divmod(g, 2)
    dt = d_bufs[t % 2]
    for ocb in range(2):
        bias_ap = cw[:, s, 2 + ocb : 3 + ocb]
        for prb in range(2):
            pl0 = PRB * prb
            banks = [ps_pool.tile([128, PRB, W], F32, name="ps") for _ in range(4)]
            if "matmuls" not in ABLATE:
                for m in range(4):
                    t = 0
                    for icb in range(2):
                        for kx in range(3):
                            nc.tensor.matmul(
                                banks[m][:],
                                w_s[:, icb, m * 3 + kx, ocb * 128 : (ocb + 1) * 128],
                                dt[:, icb, m, pl0 : pl0 + PRB, kx + 1 : kx + 1 + W],
                                start=(t == 0),
                                stop=(t == 5),
                            )
                            t += 1
            ts = [t_pool.tile([128, PRB, W], BF16, name="t") for _ in range(4)]
            if "act" not in ABLATE:
                for m in range(4):
                    nc.scalar.copy(ts[m][:], banks[m][:])
            ot = o_pool.tile([128, 2 * PRB, W], BF16, name="ot")
            if "dve_evict" not in ABLATE:
                e = e_pool.tile([128, PRB, W], BF16, name="e")
                o = e_pool.tile([128, PRB, W], BF16, name="o")
                nc.vector.scalar_tensor_tensor(
                    e[:], ts[0][:], bias_ap, ts[1][:], AluOpType.add, AluOpType.add
                )
                nc.vector.tensor_add(ot[:, 0 : 2 * PRB : 2, :], e[:], ts[2][:])
                nc.vector.scalar_tensor_tensor(
                    o[:], ts[1][:], bias_ap, ts[2][:], AluOpType.add, AluOpType.subtract
                )
                nc.vector.tensor_sub(ot[:, 1 : 2 * PRB : 2, :], o[:], ts[3][:])
            if "out_dma" not in ABLATE:
                src = ts[0] if "dve_evict" in ABLATE else ot
                r0 = 2 * (SLAB_PR * k + HS_PR * h + pl0)
                nc.sync.dma_start(
                    y_d[s, ocb * 128 : (ocb + 1) * 128,
                        r0 : r0 + (PRB if "dve_evict" in ABLATE else 2 * PRB), :],
                    src[:],
                )


def _emit_compute(nc, tc, ws_pool, t_pool, e_pool, o_pool, ps_pool,
                  x_d, y_d, cw, w_base, xp_bufs, d_bufs, zr):
    ng = 2 * NSLAB
    total = B_LOC * ng

    def w_scale(s):
        w_s = ws_pool.tile([128, 2, 12, C], BF16, name="w_s")
        for icb in range(2):
            nc.vector.tensor_scalar_mul(
                w_s[:, icb], w_base[:, icb], cw[:, s, icb : icb + 1]
            )
        return w_s

    w_cur = w_scale(0)
    w_next = None
    _emit_xdma(nc, xp_bufs, x_d, zr, 0, 0)
    _emit_transform(nc, xp_bufs, d_bufs, 0)
    for t in range(total):
        s, g = divmod(t, ng)
        k = g // 2
        if g == 0 and s > 0:
            w_cur, w_next = w_next, None
        if g % 2 == 0:
            # prefetch the next slab's rows (crossing into the next sample)
            if k + 1 < NSLAB:
                _emit_xdma(nc, xp_bufs, x_d, zr, s, k + 1)
            elif s + 1 < B_LOC:
                w_next = w_scale(s + 1)
                _emit_xdma(nc, xp_bufs, x_d, zr, s + 1, 0)
        if t + 1 < total:
            _emit_transform(nc, xp_bufs, d_bufs, t + 1)
        _emit_sets(nc, t_pool, e_pool, o_pool, ps_pool, y_d, cw, w_cur,
                   d_bufs, s, t)


def _get_nc():
    if "nc" not in _cache:
        _cache["nc"] = _build()
    return _cache["nc"]


def _make_in_maps(inputs):
    input = np.ascontiguousarray(np.asarray(inputs["input"], dtype=np.float32))
    cond = np.asarray(inputs["condition_feature"], dtype=np.float32).reshape(B, C)
    weight = np.asarray(inputs["weight"], dtype=np.float32)
    bias = np.asarray(inputs["bias"], dtype=np.float32)

    scale = 1.0 / math.sqrt(C * KH * KW)
    ws = weight * scale  # [oc, ic, ky, kx]
    W0, W1, W2 = ws[:, :, 0, :], ws[:, :, 1, :], ws[:, :, 2, :]
    G = np.stack([W0, (W0 + W1 + W2) / 2, (W0 - W1 + W2) / 2, W2])  # [m, oc, ic, kx]
    # [m, oc, ic, kx] -> [p, icb, m*3+kx, oc]
    w_host = np.ascontiguousarray(
        G.transpose(2, 0, 3, 1)          # [ic, m, kx, oc]
        .reshape(2, 128, 12, C)
        .transpose(1, 0, 2, 3)
        .astype(NPBF16)
    )
    bias_mat = np.repeat(bias, B).reshape(B, C)  # [s, oc]

    x_bf = input.astype(NPBF16)
    zr = np.zeros((128, 2, 1, W), dtype=NPBF16)
    zc = np.zeros((128, 4, HS_PR, 1), dtype=NPBF16)

    in_maps = []
    for c in range(NCORES):
        sl = slice(c * B_LOC, (c + 1) * B_LOC)
        cwm = np.empty((128, B_LOC, 4), dtype=np.float32)
        cond_c = cond[sl]  # [B_LOC, C]
        bias_c = bias_mat[sl]
        for s in range(B_LOC):
            cwm[:, s, 0] = cond_c[s, 0:128]
            cwm[:, s, 1] = cond_c[s, 128:256]
            cwm[:, s, 2] = bias_c[s, 0:128]
            cwm[:, s, 3] = bias_c[s, 128:256]
        in_maps.append(
            {"x": x_bf[sl], "w": w_host, "cw": cwm, "zr": zr, "zc": zc}
        )
    return in_maps


def kernel(input, condition_feature, weight, bias):
    in_maps = _make_in_maps(
        {
            "input": input,
            "condition_feature": condition_feature,
            "weight": weight,
            "bias": bias,
        }
    )
    nc = _get_nc()
    res = run_bass_kernel_spmd(nc, in_maps, list(range(NCORES)))
    out = np.concatenate([res.results[c]["y"] for c in range(NCORES)], axis=0)
    return np.ascontiguousarray(out.astype(np.float32))


if __name__ == "__main__":
    rng = np.random.default_rng(0)
    inputs = {
        "input": rng.standard_normal((B, C, H, W), dtype=np.float32),
        "condition_feature": rng.random((B, 1, C, 1, 1), dtype=np.float32),
        "weight": rng.standard_normal((C, C, KH, KW), dtype=np.float32),
        "bias": rng.standard_normal((C,), dtype=np.float32) * 0.1,
    }
    out = kernel(**inputs)
    print("out", out.shape, out.dtype, float(np.abs(out).max()))
